# revision 32
# baseline (speedup 1.0000x reference)
"""Causal self-attention (B=2, T=2048, C=1024, H=16) on 8 trn2 NeuronCores.

Sharding: tensor-parallel over heads — core c owns heads (2c, 2c+1).
Each core computes q/k/v for its 2 heads, runs causal attention for them,
and produces a partial c_proj output (contraction over its 128 y-features).
Partials are summed on the host.

v2 (fp8 DoubleRow): the qkv projection runs in fp8 e4m3 with an exact-ish
hi/lo split of both x and w (3 of the 4 cross products; the dropped lo*lo
term is ~2^-8 relative).  DoubleRow packs 2 contraction chunks per matmul at
0.5 cycles/row, so qkv costs 12 instructions of 256 cycles instead of 8x512
bf16 cycles (25% less PE).  Scores q^T k use a single-fp8 q against an
exact fp8-pair k — one DoubleRow matmul per key chunk at half the bf16 cost;
the ~2.7% relative score noise is additive (~0.03 absolute) and stays ~1%
on the final output after softmax.  All global scales (w pre-scaled by 32
for fp8 range, the 1/sqrt(hd) score scale) are folded into the exp's scale
argument and the host-side w_proj scaling — zero extra on-chip ops.

Device-side layouts (host pre-transposes/casts):
  xhl    [2048, 4096] fp8 x^T hi/lo chunk-interleaved: row (2c+e)*128+p =
                      (e?lo:hi) of x^T[128c+p]
  whl/wlh [128, 6144] fp8 per-core qkv weight cols, (hi,lo)/(lo,hi)
                      interleaved per 128-chunk; q/k rows de-interleaved per
                      head for rope; all scaled by 32
  wpT    [128, 1024]  bf16 w_proj rows for this core's y-features, /32
  cos/sin [128, 2048] bf16 RoPE tables (sin carries rotation sign)
Scores are computed transposed (S^T[keys, queries]); exp(x/8192) un-scales;
denominators come from a shared ones column placed *between* the two heads'
v blocks in vP ([h0 v | ones | h1 v], 130 wide) so both heads' AV matmuls
see a contiguous 65-wide rhs.  GPSIMD (Pool) takes the rope tail (fp8 q/k
writes + k hi/lo split) and the causal-mask multiplies; the PSUM->SBUF
copies for c_proj are split between ACT and DVE.
"""

import numpy as np
import ml_dtypes

B, T, C, H = 2, 2048, 1024, 16
HD = C // H          # 64
NT = B * T           # 4096
NCORES = 8
HPC = H // NCORES    # heads per core = 2
CPC = HPC * HD       # y-features per core = 128

TOK_TILE = 512       # moving-dim tile for qkv/proj matmuls and q-tiles
NJ = NT // TOK_TILE  # 8 token tiles
NCH = 128            # key-chunk size
NI = T // TOK_TILE   # 4 q-tiles per batch
BF16 = ml_dtypes.bfloat16
FP8 = ml_dtypes.float8_e4m3

WSCALE = 32.0        # fp8-range pre-scale on w_attn (both q/k and v)
EXPSCALE = 1.0 / (WSCALE * WSCALE * np.sqrt(HD))
MASKV = 240.0        # fp8e4 max; mask adds -2*240^2 -> exp(-14) ~ 9e-7

ROPE_DVE_TILES = (0, 4)   # rope on DVE for the first tiles, Pool after
N_OST_ACT = 2             # of the 8 c_proj copies per tile, how many on ACT

_CACHE = {}


def _split_waits(nc):
    """Cap sync waits at one per instruction.

    The walrus in this container rejects >1 sync-wait command on an
    instruction (seen for CTRL drains and DMA pseudo-instructions alike).
    Move all but the last wait of every instruction onto EventSemaphore
    instructions inserted just before it on the same engine.
    """
    import concourse.mybir as mybir

    n = 0
    for fn in nc.m.functions:
        for bb in fn.blocks:
            insts = bb.instructions
            out = []
            changed = False
            for inst in insts:
                si = inst.sync_info
                if si is not None and si.on_wait and len(si.on_wait) > 1:
                    waits = list(si.on_wait)
                    for w in waits[:-1]:
                        ev = mybir.InstEventSemaphore(
                            name=f"I-wsplit-{n}", ins=[], outs=[]
                        )
                        n += 1
                        ev.engine = inst.engine
                        ev.sync_info = mybir.SyncInfo(on_wait=[w], on_update=[])
                        out.append(ev)
                    si.on_wait = waits[-1:]
                    inst.sync_info = si
                    changed = True
                out.append(inst)
            if changed:
                bb.instructions = out


def _emit(nc, tc, ctx):
    import concourse.mybir as mybir
    from concourse.bass import AP as bass_AP
    from concourse.masks import make_identity

    DT = mybir.dt.bfloat16
    D8 = mybir.dt.float8e4
    F32 = mybir.dt.float32
    DR = mybir.MatmulPerfMode.DoubleRow
    Exp = mybir.ActivationFunctionType.Exp
    Copy = mybir.ActivationFunctionType.Copy
    MUL = mybir.AluOpType.mult
    ADD = mybir.AluOpType.add

    xhl_d = nc.declare_dram_parameter("xhl", [2 * C, NT], D8, isOutput=False)
    whl_d = nc.declare_dram_parameter("whl", [128, 16 * 3 * CPC], D8, isOutput=False)
    wlh_d = nc.declare_dram_parameter("wlh", [128, 16 * 3 * CPC], D8, isOutput=False)
    wpT_d = nc.declare_dram_parameter("wpT", [CPC, C], DT, isOutput=False)
    cos_d = nc.declare_dram_parameter("cos", [128, T], DT, isOutput=False)
    sin_d = nc.declare_dram_parameter("sin", [128, T], DT, isOutput=False)
    # step [128, 2, 128] fp8: -240 * (col > row), both DoubleRow slots
    dmask_d = nc.declare_dram_parameter("dmask", [128, 2 * NCH], D8, isOutput=False)
    psw_d = nc.declare_dram_parameter("psw", [128, 128], DT, isOutput=False)
    outT_d = nc.declare_dram_parameter("outT", [C, NT], DT, isOutput=True)

    const = ctx.enter_context(tc.tile_pool(name="const", bufs=1))
    xtp = ctx.enter_context(tc.tile_pool(name="xtp", bufs=2))
    work = ctx.enter_context(tc.tile_pool(name="work", bufs=5))
    esp = ctx.enter_context(tc.tile_pool(name="esp", bufs=24))
    stage = ctx.enter_context(tc.tile_pool(name="stage", bufs=2))
    psS = ctx.enter_context(tc.tile_pool(name="psS", bufs=2, space="PSUM"))
    psY = ctx.enter_context(tc.tile_pool(name="psY", bufs=2, space="PSUM"))
    psA = ctx.enter_context(tc.tile_pool(name="psA", bufs=2, space="PSUM"))

    # ---- persistent SBUF tensors; first x chunks and the first half of the
    # qkv weights are DMA'd ahead of everything else so the PE starts ASAP ----
    def load_xt(j, split=False, tile=None):
        # xt [128, 8 chunks, 2 (hi,lo), 512 tokens]
        xt = tile if tile is not None else xtp.tile(
            [128, 8, 2, TOK_TILE], D8, tag="xt", name=f"xt{j}"
        )
        src = xhl_d.rearrange("(a b p) n -> p a b n", p=128, b=2)[
            :, :, :, TOK_TILE * j : TOK_TILE * (j + 1)
        ]
        if split:
            # chunk pairs on the ACT DMA queue: halves SP seq serialization
            # at startup and matches the hh chunk-pair consumption order
            for cp in range(4):
                nc.scalar.dma_start(
                    out=xt[:, 2 * cp : 2 * cp + 2], in_=src[:, 2 * cp : 2 * cp + 2]
                )
        else:
            nc.sync.dma_start(out=xt, in_=src)
        return xt

    # w tiles are f-major: [128, 3 (q|k|v), 8 chunks, 2 (hi,lo), 128] so each
    # f's weights arrive in one small contiguous DMA
    whl = const.tile([128, 3, 8, 2, CPC], D8, tag="whl")
    wlh = const.tile([128, 3, 8, 2, CPC], D8, tag="wlh")
    whl_src = whl_d.rearrange("p (f a b m) -> p f a b m", f=3, a=8, b=2)
    wlh_src = wlh_d.rearrange("p (f a b m) -> p f a b m", f=3, a=8, b=2)
    cos_sb = const.tile([128, T], DT, tag="cos")
    sin_sb = const.tile([128, T], DT, tag="sin")
    psw_sb = const.tile([128, 128], DT, tag="psw")
    dmask_sb = const.tile([128, 2, NCH], D8, tag="dmask")
    nc.sync.dma_start(out=whl[:, 0], in_=whl_src[:, 0])
    nc.sync.dma_start(out=wlh[:, 0], in_=wlh_src[:, 0])
    xt0 = load_xt(0, split=True)
    nc.sync.dma_start(out=whl[:, 1], in_=whl_src[:, 1])
    nc.sync.dma_start(out=wlh[:, 1], in_=wlh_src[:, 1])
    nc.sync.dma_start(out=psw_sb, in_=psw_d[:])
    nc.sync.dma_start(out=dmask_sb, in_=dmask_d.rearrange("p (a n) -> p a n", a=2))
    nc.sync.dma_start(out=cos_sb[:, 0:1024], in_=cos_d[:, 0:1024])
    nc.sync.dma_start(out=sin_sb[:, 0:1024], in_=sin_d[:, 0:1024])
    nc.sync.dma_start(out=whl[:, 2], in_=whl_src[:, 2])
    nc.sync.dma_start(out=wlh[:, 2], in_=wlh_src[:, 2])
    xt4 = load_xt(4)
    nc.sync.dma_start(out=cos_sb[:, 1024:T], in_=cos_d[:, 1024:T])
    nc.sync.dma_start(out=sin_sb[:, 1024:T], in_=sin_d[:, 1024:T])
    ident = const.tile([128, 128], DT, tag="ident")
    make_identity(nc, ident)
    # fp8 +448 identity, both DoubleRow slots, for the causal-mask matmul
    mid8 = const.tile([128, 2, 128], D8, tag="mid8")
    nc.gpsimd.tensor_scalar_mul(mid8[:, 0, :], ident, MASKV)
    nc.gpsimd.tensor_scalar_mul(mid8[:, 1, :], ident, MASKV)
    # warm the ACT exp table while phase 1 runs
    warm = const.tile([128, 1], F32, tag="warm")
    nc.vector.memset(warm, 0.0)
    nc.scalar.activation(warm, warm, Exp)
    wp_sb = const.tile([128, C], DT, tag="wp")
    nc.sync.dma_start(out=wp_sb, in_=wpT_d[:])

    q8_sb = const.tile([128, NT], D8, tag="q8")
    khl_sb = const.tile([128, 2, NT], D8, tag="khl")
    yT_sb = const.tile([128, NT], DT, tag="yT")
    # vP [tok 128, chunk, h0 v (0:64) | ones (64) | h1 v (65:129)]
    vP = const.tile([128, NT // NCH, 2 * HD + 2], DT, tag="vP")
    nc.vector.memset(vP[:, :, HD : HD + 1], 1.0)

    def q0s(h, csl):
        """q8 [64, 2(dup), cols] with a 0-stride middle dim for DoubleRow."""
        base = q8_sb[HD * h : HD * (h + 1), csl]
        return bass_AP(
            tensor=q8_sb.tensor, offset=base.offset,
            ap=[base.ap[0], [0, 2], base.ap[1]],
        )

    def qkv_mms(j, xt, f, acc, seq):
        """The 12 DoubleRow matmuls for one f: 4 hi*hi chunk-pairs + 8
        cross (lo,hi), as (lhsT, rhs) pairs appended to seq."""
        for u in range(4):
            seq.append(
                (f, acc,
                 whl[:, f, 2 * u : 2 * u + 2, 0, :],
                 xt[:, 2 * u : 2 * u + 2, 0, :])
            )
            for c in (2 * u, 2 * u + 1):
                seq.append((f, acc, wlh[:, f, c, :, :], xt[:, c, :, :]))

    def qkv_gen(j, xt=None):
        """Project tokens [512j, 512j+512): q (+rope, fp8) into q8, k into
        khl hi/lo, v into vP.  Yields after each PE matmul so the caller can
        interleave this PE work into ACT-bound stretches of attention.
        q and k matmuls are interleaved chunk-major so the first tile can
        start before all its x chunks have landed; the rope partition-swap
        is a PE matmul against a permutation matrix (no DMA round-trip)."""
        jsl = slice(TOK_TILE * j, TOK_TILE * (j + 1))
        csl = slice(TOK_TILE * (j % NI), TOK_TILE * (j % NI) + TOK_TILE)
        eng = nc.vector if j in ROPE_DVE_TILES else nc.gpsimd
        if xt is None:
            xt = load_xt(j)
        raw = work.tile([128, 2, TOK_TILE], DT, tag="raw", name=f"raw{j}", bufs=3)
        sws = work.tile([128, 2, TOK_TILE], DT, tag="sws", name=f"sws{j}", bufs=3)
        accq = psA.tile([128, TOK_TILE], F32, tag="ps", name=f"qkv{j}_0")
        acck = psA.tile([128, TOK_TILE], F32, tag="ps", name=f"qkv{j}_1")
        seq = []
        for u in range(4):
            for f, acc in ((0, accq), (1, acck)):
                seq.append(
                    (f, acc,
                     whl[:, f, 2 * u : 2 * u + 2, 0, :],
                     xt[:, 2 * u : 2 * u + 2, 0, :])
                )
            for c in (2 * u, 2 * u + 1):
                for f, acc in ((0, accq), (1, acck)):
                    seq.append((f, acc, wlh[:, f, c, :, :], xt[:, c, :, :]))
        nf = [0, 0]
        for (f, acc, lhsT, rhs) in seq:
            nf[f] += 1
            nc.tensor.matmul(
                acc, lhsT=lhsT, rhs=rhs,
                start=(nf[f] == 1), stop=(nf[f] == 12), perf_mode=DR,
            )
            yield
            if nf[f] == 12:
                # rope swap: sws_psum[p] = raw[p^32] via permutation matmul
                nc.vector.tensor_copy(raw[:, f, :], acc)
                swp = psA.tile([128, TOK_TILE], F32, tag="ps", name=f"swp{j}_{f}")
                nc.tensor.matmul(swp, lhsT=psw_sb, rhs=raw[:, f, :],
                                 start=True, stop=True)
                nc.vector.tensor_mul(sws[:, f, :], swp, sin_sb[:, csl])
        # rope: out = raw*cos + swap32(raw)*sin (sign folded into sin)
        cos2 = bass_AP(
            tensor=cos_sb.tensor, offset=cos_sb[:, csl].offset,
            ap=[cos_sb.ap[0], [0, 2], cos_sb[:, csl].ap[1]],
        )
        eng.tensor_mul(raw, raw, cos2)
        # q: straight to fp8; k: bf16 then exact hi/lo fp8 split
        eng.tensor_add(q8_sb[:, jsl], raw[:, 0, :], sws[:, 0, :])
        kbf = work.tile([128, TOK_TILE], DT, tag="kbf", name=f"kbf{j}", bufs=2)
        eng.tensor_add(kbf, raw[:, 1, :], sws[:, 1, :])
        eng.tensor_copy(khl_sb[:, 0, jsl], kbf)
        eng.scalar_tensor_tensor(
            khl_sb[:, 1, jsl], khl_sb[:, 0, jsl], -1.0, kbf, MUL, ADD
        )
        for f in (2,):
            acc = psA.tile([128, TOK_TILE], F32, tag="ps", name=f"qkv{j}_2")
            vseq = []
            qkv_mms(j, xt, 2, acc, vseq)
            for n_mm, (_, _, lhsT, rhs) in enumerate(vseq):
                nc.tensor.matmul(
                    acc, lhsT=lhsT, rhs=rhs,
                    start=(n_mm == 0), stop=(n_mm == 11), perf_mode=DR,
                )
                yield
            vsb = work.tile([128, TOK_TILE], DT, tag="vsb", name=f"vsb{j}")
            nc.vector.tensor_copy(vsb, acc)
            for s in range(TOK_TILE // NCH):
                pst = psA.tile([128, 128], DT, tag="ps", name=f"vt{j}_{s}")
                nc.tensor.transpose(pst, vsb[:, 128 * s : 128 * s + 128], ident)
                ch = (TOK_TILE * j) // NCH + s
                dst = vP[:, ch, :]
                out2 = bass_AP(
                    tensor=vP.tensor, offset=dst.offset,
                    ap=[dst.ap[0], [HD + 1, 2], [1, HD]],
                )
                nc.vector.tensor_copy(
                    out2,
                    pst.rearrange("p (a b) -> p a b", a=2),
                )

    def s_gen(b, i, pairs):
        """Score chunks S^T[keys, queries] + exp for q-tile (b, i); fills
        `pairs` and yields after each chunk pair.  One DoubleRow matmul per
        key chunk: (k_hi,k_lo) x (q8,q8); diagonal chunks only compute the
        needed column range."""
        tok0 = T * b
        m0 = tok0 + TOK_TILE * i
        nch = 4 * (i + 1)
        for h in range(HPC):
            pairs.append([None] * (nch // 2))
        for u in range(nch // 2):
            for h in range(HPC):
                hsl = slice(HD * h, HD * (h + 1))
                ssp = psS.tile([128, 2 * TOK_TILE], F32, tag="ps", name=f"s{b}{i}{u}{h}")
                for idx in range(2):
                    jn = 2 * u + idx
                    n0 = tok0 + NCH * jn
                    k = jn - 4 * i
                    c0 = NCH * max(k, 0)
                    diag = k >= 0
                    nc.tensor.matmul(
                        ssp[:, TOK_TILE * idx + c0 : TOK_TILE * (idx + 1)],
                        lhsT=khl_sb[hsl, :, n0 : n0 + NCH],
                        rhs=q0s(h, slice(m0 + c0, m0 + TOK_TILE)),
                        start=True, stop=not diag, perf_mode=DR,
                        skip_group_check=diag,
                    )
                    if diag:
                        # causal mask: accumulate -2*448*448 onto keys>queries
                        # of the diagonal chunk via a step x identity matmul
                        nc.tensor.matmul(
                            ssp[:, TOK_TILE * idx + c0 : TOK_TILE * idx + c0 + NCH],
                            lhsT=dmask_sb,
                            rhs=mid8,
                            start=False, stop=True, perf_mode=DR,
                            skip_group_check=True,
                        )
                es = esp.tile(
                    [128, 2 * TOK_TILE], DT, tag=f"es{h}", name=f"es{b}{i}{u}{h}"
                )
                k1 = 2 * u + 1 - 4 * i
                if k1 <= 0:
                    nc.scalar.activation(es, ssp, Exp, scale=EXPSCALE)
                else:
                    for idx in range(2):
                        k = 2 * u + idx - 4 * i
                        c0 = TOK_TILE * idx + NCH * max(k, 0)
                        c1 = TOK_TILE * (idx + 1)
                        nc.scalar.activation(
                            es[:, c0:c1], ssp[:, c0:c1], Exp, scale=EXPSCALE
                        )
                pairs[h][u] = es
            yield

    def emit_AV_proj(b, i, pairs, pump_s, pump_q, tail=False):
        """attn @ v' for q-tile (b, i), normalize, transpose into yT, then the
        c_proj slice for these 512 tokens and its output DMA.  With tail=True
        the c_proj is done in two 256-token halves, each started as soon as
        its two yT sub-blocks are ready (shorter serial tail)."""
        tok0 = T * b
        m0 = tok0 + TOK_TILE * i
        yps = []
        for h in range(HPC):
            yp = psY.tile([128, 4, HD + 1], F32, tag="ps", name=f"y{b}{i}{h}")
            # rhs: h0 reads [v|ones] (cols 0:65), h1 [ones|v] (cols 64:129)
            for sm in range(4):
                njn = 4 * i + sm + 1
                for jn in range(njn):
                    es = pairs[h][jn // 2]
                    base = TOK_TILE * (jn % 2)
                    nc.tensor.matmul(
                        yp[:, sm, :],
                        lhsT=es[:, base + NCH * sm : base + NCH * (sm + 1)],
                        rhs=vP[:, (tok0 // NCH) + jn, HD * h : HD * h + HD + 1],
                        start=(jn == 0),
                        stop=(jn == njn - 1),
                    )
                pump_s(1)
                pump_q(4)
            yps.append(yp)

        def norm_sm(sm):
            ypk = work.tile([128, 128], DT, tag="ypk", name=f"ypk{b}{i}{sm}")
            for h in range(HPC):
                recip = work.tile([128, 1], F32, tag="recip", name=f"rc{b}{i}{sm}{h}")
                den_c = HD if h == 0 else 0
                ysl = slice(0, HD) if h == 0 else slice(1, HD + 1)
                nc.vector.reciprocal(recip, yps[h][:, sm, den_c : den_c + 1])
                nc.vector.tensor_scalar_mul(
                    ypk[:, HD * h : HD * (h + 1)], yps[h][:, sm, ysl], recip
                )
            pst = psA.tile([128, 128], DT, tag="ps", name=f"yt{b}{i}{sm}")
            nc.tensor.transpose(pst, ypk, ident)
            nc.vector.tensor_copy(yT_sb[:, m0 + NCH * sm : m0 + NCH * (sm + 1)], pst)
            pump_q(2)

        def proj(msl, cols, part):
            for og in range(2):
                ost = stage.tile(
                    [128, 4, cols], DT, tag=f"ost{part}", name=f"ost{b}{i}{og}{part}"
                )
                for oi in range(4):
                    ot = 4 * og + oi
                    osp = psA.tile([128, cols], F32, tag="ps", name=f"o{b}{i}{ot}{part}")
                    nc.tensor.matmul(
                        osp,
                        lhsT=wp_sb[:, 128 * ot : 128 * (ot + 1)],
                        rhs=yT_sb[:, msl],
                        start=True, stop=True,
                    )
                    if ot < N_OST_ACT:
                        nc.scalar.activation(ost[:, oi, :], osp, Copy)
                    else:
                        nc.vector.tensor_copy(ost[:, oi, :], osp)
                    pump_q(2)
                nc.sync.dma_start(
                    out=outT_d.rearrange("(a p) n -> p a n", p=128)[
                        :, 4 * og : 4 * og + 4, msl
                    ],
                    in_=ost,
                )
                pump_s(1)

        if not tail:
            for sm in range(4):
                norm_sm(sm)
            proj(slice(m0, m0 + TOK_TILE), TOK_TILE, "")
        else:
            norm_sm(0)
            norm_sm(1)
            proj(slice(m0, m0 + 2 * NCH), 2 * NCH, "a")
            norm_sm(2)
            norm_sm(3)
            proj(slice(m0 + 2 * NCH, m0 + TOK_TILE), 2 * NCH, "b")

    # ---- pipeline ----
    # Two paced work queues are drip-fed between the attention matmuls of
    # earlier tiles: qkv matmul units (never stall; main PE filler) and score
    # chunk-pairs for upcoming tiles (at most ~one per AV sub-chain, matching
    # the rate ACT drains them through exp).
    tiles = [(0, 0), (1, 0), (0, 1), (1, 1), (0, 2), (1, 2), (0, 3), (1, 3)]
    qkv_js = [4, 1, 5, 2, 6, 3, 7]
    qkv_gens = [qkv_gen(4, xt4)] + [qkv_gen(j) for j in qkv_js[1:]]
    qkv_done = [1]  # count of fully-emitted qkv tiles (incl. eager j=0)
    pair_store = {t: [] for t in tiles}
    s_gens = [s_gen(b, i, pair_store[(b, i)]) for (b, i) in tiles]
    s_done = [0]  # count of fully-emitted s tiles
    # s tile k may only emit once the first k+1 qkv tiles are done
    s_need = list(range(1, 9))

    # eager qkv(0): emit q/k projection + rope, then scores (0,0) right away
    # so the first exp reaches ACT as early as possible, then v
    g0 = qkv_gen(0, xt0)
    for _ in range(25):
        next(g0)
    for _ in s_gens[0]:
        pass
    s_done[0] = 1
    for _ in g0:
        pass

    def pump_q(n):
        done = 0
        while qkv_done[0] <= len(qkv_js) and done < n:
            g = qkv_gens[qkv_done[0] - 1]
            try:
                next(g)
                done += 1
            except StopIteration:
                qkv_done[0] += 1

    def pump_s(n):
        done = 0
        while s_done[0] < len(tiles) and done < n:
            k = s_done[0]
            if qkv_done[0] < s_need[k]:
                pump_q(1)
                if qkv_done[0] < s_need[k]:
                    return
                continue
            try:
                next(s_gens[k])
                done += 1
            except StopIteration:
                # count the tile boundary as work: never roll into the next
                # tile's scores from a pump (its rope may not be emitted yet)
                s_done[0] += 1
                done += 1

    def pump_none(n):
        pass

    for k, (b, i) in enumerate(tiles):
        while s_done[0] <= k:
            pump_s(1)
            pump_q(1)
        # per cycle: [qkv for s(k+2)] [scores k+1] [AV k].  The qkv runs two
        # cycles ahead of its score consumer so its rope chain (Pool-serial,
        # ~6us) has a full AV cycle to finish; exp(k+1) then runs on ACT
        # while the PE grinds AV(k) without in-order-queue stalls.
        if k + 2 < len(tiles):
            while qkv_done[0] < s_need[k + 2]:
                pump_q(8)
        if k + 1 < len(tiles):
            while s_done[0] <= k + 1:
                pump_s(1)
        emit_AV_proj(b, i, pair_store[(b, i)], pump_none, pump_q,
                     tail=(k == len(tiles) - 1))
    while qkv_done[0] <= len(qkv_js):
        pump_q(100)


def _build_nc(split_waits=True):
    from contextlib import ExitStack

    import concourse.bass as bass
    import concourse.tile as tile

    nc = bass.Bass("TRN2", target_bir_lowering=False, debug=False, num_devices=NCORES)
    with ExitStack() as ctx:
        tc = ctx.enter_context(tile.TileContext(nc))
        _emit(nc, tc, ctx)
    if split_waits:
        # CoreSim's race detector can't digest the inserted EventSemaphores;
        # build with split_waits=False when simulating.
        _split_waits(nc)
    return nc


def _hilo(a):
    """Split float array into fp8 hi + residual lo (stored fp8)."""
    hi = a.astype(FP8)
    lo = (a - hi.astype(np.float32)).astype(FP8)
    return hi, lo


def _prep_inputs(x, w_attn, w_proj, freqs_cos, freqs_sin):
    x = np.asarray(x, np.float32)
    w_attn = np.asarray(w_attn, np.float32)
    w_proj = np.asarray(w_proj, np.float32)
    fc = np.asarray(freqs_cos, np.float32)
    fs = np.asarray(freqs_sin, np.float32)

    perm = np.concatenate([np.arange(0, HD, 2), np.arange(1, HD, 2)])
    xT = np.ascontiguousarray(x.reshape(NT, C).T)  # [C, NT] fp32
    xh, xl = _hilo(xT)
    # xhl row (2c+e)*128+p = (e ? lo : hi)[128c+p]
    xhl = np.empty((2 * C, NT), FP8)
    xv = xhl.reshape(8, 2, 128, NT)
    xv[:, 0] = xh.reshape(8, 128, NT)
    xv[:, 1] = xl.reshape(8, 128, NT)

    pair = np.arange(128) % 32
    COS = fc[:, pair].T.copy()  # [128, T]
    SIN = fs[:, pair].T.copy()
    sign = np.where((np.arange(128) % 64) < 32, -1.0, 1.0).astype(np.float32)
    SIN = SIN * sign[:, None]
    step = (-MASKV * (np.arange(NCH)[None, :] > np.arange(128)[:, None])).astype(FP8)
    psw = (np.arange(128)[:, None] == (np.arange(128)[None, :] ^ 32)).astype(BF16)
    shared = {
        "xhl": xhl,
        "cos": COS.astype(BF16),
        "sin": SIN.astype(BF16),
        "dmask": np.ascontiguousarray(
            np.stack([step, step], axis=1).reshape(128, 2 * NCH)
        ),
        "psw": psw,
    }

    in_maps = []
    for core in range(NCORES):
        heads = range(HPC * core, HPC * core + HPC)
        rows_q = np.concatenate([h * HD + perm for h in heads])
        rows_k = np.concatenate([C + h * HD + perm for h in heads])
        rows_v = np.concatenate([2 * C + h * HD + np.arange(HD) for h in heads])
        wT = np.ascontiguousarray(
            np.concatenate(
                [w_attn[rows_q], w_attn[rows_k], w_attn[rows_v]], axis=0
            ).T
        ) * WSCALE  # [1024, 384] fp32
        wh, wl = _hilo(wT)
        # whl: [128 p, 3 f, 8 c, 2 (hi,lo), 128]; wlh: (lo,hi)
        whl = np.empty((128, 3, 8, 2, CPC), FP8)
        wlh = np.empty((128, 3, 8, 2, CPC), FP8)
        # wh [1024, 384] -> [8 c, 128 p, 3 f, 128 m]
        whl[:, :, :, 0] = wh.reshape(8, 128, 3, CPC).transpose(1, 2, 0, 3)
        whl[:, :, :, 1] = wl.reshape(8, 128, 3, CPC).transpose(1, 2, 0, 3)
        wlh[:, :, :, 0] = whl[:, :, :, 1]
        wlh[:, :, :, 1] = whl[:, :, :, 0]
        cols_v = np.concatenate([h * HD + np.arange(HD) for h in heads])
        wpT = np.ascontiguousarray(w_proj[:, cols_v].T / WSCALE).astype(BF16)
        in_maps.append(
            {
                **shared,
                "whl": whl.reshape(128, -1),
                "wlh": wlh.reshape(128, -1),
                "wpT": wpT,
            }
        )
    return in_maps


def _make_runner():
    """Compile the kernel once and return a reusable multi-core runner.

    Mirrors bass2jax.run_bass_via_pjrt's shard_map path, but keeps the jitted
    executable so repeat kernel() calls skip retracing/compile-cache lookups.
    """
    import jax
    import numpy as _np
    from jax.sharding import Mesh, PartitionSpec
    from jax.experimental.shard_map import shard_map

    import concourse.mybir as mybir
    from concourse import bass2jax

    nc = _build_nc()
    bass2jax.install_neuronx_cc_hook()

    partition_name = nc.partition_id_tensor.name if nc.partition_id_tensor else None
    in_names, out_names, out_avals, zero_shapes = [], [], [], []
    for alloc in nc.m.functions[0].allocations:
        if not isinstance(alloc, mybir.MemoryLocationSet):
            continue
        name = alloc.memorylocations[0].name
        if alloc.kind == "ExternalInput":
            if name != partition_name:
                in_names.append(name)
        elif alloc.kind == "ExternalOutput":
            shape = tuple(alloc.tensor_shape)
            dtype = mybir.dt.np(alloc.dtype)
            out_names.append(name)
            out_avals.append(jax.core.ShapedArray(shape, dtype))
            zero_shapes.append((shape, dtype))
    n_params = len(in_names)
    n_outs = len(out_avals)
    all_names = in_names + out_names + ([partition_name] if partition_name else [])
    donate = tuple(range(n_params, n_params + n_outs))

    def _body(*args):
        operands = list(args)
        if partition_name is not None:
            operands.append(bass2jax.partition_id_tensor())
        outs = bass2jax._bass_exec_p.bind(
            *operands,
            out_avals=tuple(out_avals),
            in_names=tuple(all_names),
            out_names=tuple(out_names),
            lowering_input_output_aliases=(),
            sim_require_finite=True,
            sim_require_nnan=True,
            nc=nc,
        )
        return tuple(outs)

    devices = jax.devices()[:NCORES]
    mesh = Mesh(_np.asarray(devices), ("core",))
    sharded = jax.jit(
        shard_map(
            _body,
            mesh=mesh,
            in_specs=(PartitionSpec("core"),) * (n_params + n_outs),
            out_specs=(PartitionSpec("core"),) * n_outs,
            check_rep=False,
        ),
        donate_argnums=donate,
        keep_unused=True,
    )

    def run(in_maps):
        concat_in = [
            np.concatenate([np.asarray(m[name]) for m in in_maps], axis=0)
            for name in in_names
        ]
        concat_zeros = [
            np.zeros((NCORES * s[0], *s[1:]), dt) for (s, dt) in zero_shapes
        ]
        out_arrs = sharded(*concat_in, *concat_zeros)
        return [
            {
                name: np.asarray(out_arrs[i]).reshape(
                    NCORES, *out_avals[i].shape
                )[c]
                for i, name in enumerate(out_names)
            }
            for c in range(NCORES)
        ]

    return run


def kernel(x, w_attn, w_proj, freqs_cos, freqs_sin):
    if "runner" not in _CACHE:
        _CACHE["runner"] = _make_runner()
    in_maps = _prep_inputs(x, w_attn, w_proj, freqs_cos, freqs_sin)
    results = _CACHE["runner"](in_maps)
    out = np.zeros((C, NT), np.float64)
    for r in results:
        out += r["outT"].astype(np.float64)
    return out.T.reshape(B, T, C).astype(np.float32)
